# revision 15
# baseline (speedup 1.0000x reference)
"""Trainium2 Bass kernel for GQA attention block (B=2, S=2048, D=4096, 32 q heads,
8 kv heads, rope, causal softmax, output projection).

Sharding: 8 cores = 2 batches x 4 kv-head-groups. Core i handles batch i//4 and
q heads 8*(i%4)..8*(i%4)+7 (kv heads 2*(i%4), 2*(i%4)+1). Each core computes a
partial [S, D] output (its heads' contribution through wo); the host sums the 4
partials per batch.

Compute runs in bf16 on the TensorEngine (fp32 PSUM accumulation). RoPE is
applied with the head dims de-interleaved (even dims in partitions 0:63, odd in
64:127) so the pair arithmetic is partition-aligned; wq/wk columns are permuted
identically on the host, which leaves all dot products unchanged. Scores are
computed transposed (S^T[k,q]) so the probs feed the PV matmul directly; softmax
skips the max subtraction (scores are bounded ~ +-11 for this distribution) and
the row sum comes from a ones-matmul on the PE, replicated across partitions.
"""

import numpy as np
import ml_dtypes

B, S, D = 2, 2048, 4096
NH, NKV, HD = 32, 8, 128
QH = 8          # q heads per core
KVH = 2         # kv heads per core
NCHUNK = 4      # seq chunks of 512
CW = 512        # chunk width
KT = 32         # k-tiles over D
ST = 16         # seq tiles of 128
ISQ = 1.0 / np.sqrt(HD)
THETA = 10000.0
NEG = -1e10

_BF16 = ml_dtypes.bfloat16

LAST_INFO = {}


def _build(trace=False):
    import concourse.bass as bass
    import concourse.mybir as mybir
    from concourse import bacc
    from concourse.tile import TileContext

    f32 = mybir.dt.float32
    bf16 = mybir.dt.bfloat16
    AF = mybir.ActivationFunctionType

    nc = bacc.Bacc("TRN2", target_bir_lowering=False, debug=False, num_devices=8)

    xt_d = nc.dram_tensor("xt", [NCHUNK, 128, KT, CW], bf16, kind="ExternalInput")
    wq_d = nc.dram_tensor("wq", [2, KT // 2, 128, 1024], bf16, kind="ExternalInput")
    wk_d = nc.dram_tensor("wk", [128, KT, 256], bf16, kind="ExternalInput")
    wv_d = nc.dram_tensor("wv", [128, KT, 256], bf16, kind="ExternalInput")
    wo_d = nc.dram_tensor("wo", [8, 128, 8, 512], bf16, kind="ExternalInput")
    cos_d = nc.dram_tensor("cos2", [128, S], bf16, kind="ExternalInput")
    sin_d = nc.dram_tensor("sin2", [128, S], bf16, kind="ExternalInput")
    mask_d = nc.dram_tensor("maskt", [128, 128], f32, kind="ExternalInput")
    out_d = nc.dram_tensor("out", [S, D], f32, kind="ExternalOutput")

    with TileContext(nc) as tc:
        with (
            tc.tile_pool(name="singles", bufs=1) as singles,
            tc.tile_pool(name="xtp", bufs=1) as xtp,
            tc.tile_pool(name="wqp", bufs=4) as wqp,
            tc.tile_pool(name="wop", bufs=3) as wop,
            tc.tile_pool(name="qtp", bufs=2) as qtp,
            tc.tile_pool(name="otp", bufs=2) as otp,
            tc.tile_pool(name="esp", bufs=6) as esp,
            tc.tile_pool(name="rtp", bufs=6) as rtp,
            tc.tile_pool(name="rcp", bufs=2) as rcp,
            tc.tile_pool(name="obp", bufs=4) as obp,
            tc.tile_pool(name="psacc", bufs=5, space="PSUM") as psacc,
            tc.tile_pool(name="pss", bufs=3, space="PSUM") as pss,
        ):
            xt_first = xtp.tile([128, KT, CW], bf16, tag="xt", name="xt_first")
            nc.sync.dma_start(out=xt_first, in_=xt_d[0, :, :, :])
            wk_sb = singles.tile([128, KT, 256], bf16, tag="wk")
            nc.sync.dma_start(out=wk_sb, in_=wk_d[:, :, :])
            wv_sb = singles.tile([128, KT, 256], bf16, tag="wv")
            nc.sync.dma_start(out=wv_sb, in_=wv_d[:, :, :])
            cos_sb = singles.tile([128, S], bf16, tag="cos")
            nc.sync.dma_start(out=cos_sb, in_=cos_d[:, :])
            sin_sb = singles.tile([128, S], bf16, tag="sin")
            nc.sync.dma_start(out=sin_sb, in_=sin_d[:, :])
            mask_sb = singles.tile([128, 128], f32, tag="mask")
            nc.sync.dma_start(out=mask_sb, in_=mask_d[:, :])
            ones_sb = singles.tile([128, 128], bf16, tag="ones")
            nc.vector.memset(ones_sb, 1.0)
            kt_sb = singles.tile([128, KVH, S], bf16, tag="kt")
            v_sb = singles.tile([128, ST, KVH, 128], bf16, tag="v")

            def rope(ps_in, out_ap, j):
                """ps_in: [128, CW] fp32 psum (de-interleaved proj block).
                out_ap: [128, CW] bf16 view <- rope result. The ACT copy frees
                the psum slot immediately; rope math runs in bf16 (DVE 2x)."""
                sl = slice(CW * j, CW * (j + 1))
                qraw = rtp.tile([128, CW], bf16, tag="rt")
                nc.scalar.copy(out=qraw, in_=ps_in)
                tA = rtp.tile([128, CW], bf16, tag="rt")
                tB = rtp.tile([128, CW], bf16, tag="rt")
                # tA: top = x0*cos, bot = x0*sin (inputs base 0)
                nc.vector.tensor_mul(tA[0:64, :], qraw[0:64, :], cos_sb[0:64, sl])
                nc.vector.tensor_mul(tA[64:128, :], qraw[0:64, :], sin_sb[0:64, sl])
                # tB: top = x1*sin, bot = x1*cos (inputs base 64)
                nc.vector.tensor_mul(tB[0:64, :], qraw[64:128, :], sin_sb[64:128, sl])
                nc.vector.tensor_mul(tB[64:128, :], qraw[64:128, :], cos_sb[64:128, sl])
                nc.vector.tensor_sub(out_ap[0:64, :], tA[0:64, :], tB[0:64, :])
                nc.vector.tensor_add(out_ap[64:128, :], tA[64:128, :], tB[64:128, :])

            for j in range(NCHUNK):
                if j == 0:
                    xt_t = xt_first
                else:
                    xt_t = xtp.tile([128, KT, CW], bf16, tag="xt", name=f"xt{j}")
                    nc.sync.dma_start(out=xt_t, in_=xt_d[j, :, :, :])

                # ---- K projection ----
                pk = [psacc.tile([128, CW], f32, tag="acc", name=f"pk{j}_{g}") for g in range(KVH)]
                for k in range(KT):
                    st, sp = (k == 0), (k == KT - 1)
                    for g in range(KVH):
                        nc.tensor.matmul(
                            pk[g], lhsT=wk_sb[:, k, g * 128:(g + 1) * 128],
                            rhs=xt_t[:, k, :], start=st, stop=sp)
                for g in range(KVH):
                    rope(pk[g], kt_sb[:, g, CW * j:CW * (j + 1)], j)

                # ---- Q passes interleaved with V halves (ropes hide under MMs) ----
                qt_t = qtp.tile([128, QH, CW], bf16, tag="qt")

                def q_pass(p):
                    pq = [psacc.tile([128, CW], f32, tag="acc", name=f"pq{j}_{p}_{m}")
                          for m in range(4)]
                    for kp in range(KT // 2):
                        wq_t = wqp.tile([128, 1024], bf16, tag="wq", name=f"wq{j}_{p}_{kp}")
                        nc.sync.dma_start(out=wq_t, in_=wq_d[p, kp, :, :])
                        for k01 in range(2):
                            k = 2 * kp + k01
                            for mm in range(4):
                                nc.tensor.matmul(
                                    pq[mm],
                                    lhsT=wq_t[:, k01 * 512 + mm * 128:k01 * 512 + (mm + 1) * 128],
                                    rhs=xt_t[:, k, :], start=(k == 0), stop=(k == KT - 1))
                    for mm in range(4):
                        rope(pq[mm], qt_t[:, 4 * p + mm, :], j)

                def v_pass(half):
                    pv = [psacc.tile([128, CW], f32, tag="acc", name=f"pv{j}_{half}_{t}")
                          for t in range(2)]
                    for k in range(KT):
                        st, sp = (k == 0), (k == KT - 1)
                        for t in range(2):
                            tt = 2 * half + t
                            nc.tensor.matmul(
                                pv[t][:, 0:256], lhsT=xt_t[:, k, tt * 128:(tt + 1) * 128],
                                rhs=wv_sb[:, k, :], start=st, stop=sp)
                    for t in range(2):
                        nc.scalar.copy(out=v_sb[:, 4 * j + 2 * half + t, :, :],
                                       in_=pv[t][:, 0:256])

                q_pass(0)
                v_pass(0)
                q_pass(1)
                v_pass(1)

                # ---- attention for this chunk's queries ----
                ot_t = otp.tile([128, QH, CW], bf16, tag="ot")
                nk = 4 * j + 4
                for h in range(QH):
                    g = h // 4
                    po = psacc.tile([128, CW], f32, tag="acc")
                    pr = psacc.tile([128, CW], f32, tag="acc")
                    for i in range(nk):
                        off = max(0, 128 * (i - 4 * j))
                        ps = pss.tile([128, CW], f32, tag="s")
                        nc.tensor.matmul(
                            ps[:, off:], lhsT=kt_sb[:, g, 128 * i:128 * (i + 1)],
                            rhs=qt_t[:, h, off:], start=True, stop=True)
                        if i >= 4 * j:
                            nc.vector.tensor_add(
                                ps[:, off:off + 128], ps[:, off:off + 128], mask_sb)
                        es = esp.tile([128, CW], bf16, tag="es")
                        nc.scalar.activation(es[:, off:], ps[:, off:], AF.Exp, scale=ISQ)
                        nc.tensor.matmul(
                            po[:, off:], lhsT=v_sb[:, i, g, :], rhs=es[:, off:],
                            start=(i == 0), stop=(i == nk - 1))
                        nc.tensor.matmul(
                            pr[:, off:], lhsT=ones_sb, rhs=es[:, off:],
                            start=(i == 0), stop=(i == nk - 1))
                    rc = rcp.tile([128, CW], f32, tag="rc")
                    nc.vector.reciprocal_approx_fast(out=rc, in_=pr)
                    nc.vector.tensor_mul(ot_t[:, h, :], po, rc)

                # ---- output projection for this chunk's rows ----
                wo_tiles = []
                for n in range(8):
                    wo_t = wop.tile([128, 8, 512], bf16, tag="wo", name=f"wo{j}_{n}")
                    nc.sync.dma_start(out=wo_t, in_=wo_d[n, :, :, :])
                    wo_tiles.append(wo_t)
                for n in range(8):
                    wo_t = wo_tiles[n]
                    for t in range(4):
                        pw = psacc.tile([128, 512], f32, tag="acc")
                        for hb in range(QH):
                            nc.tensor.matmul(
                                pw, lhsT=ot_t[:, hb, 128 * t:128 * (t + 1)],
                                rhs=wo_t[:, hb, :], start=(hb == 0), stop=(hb == QH - 1))
                        ob = obp.tile([128, 512], f32, tag="ob")
                        nc.vector.tensor_copy(ob, pw)
                        nc.sync.dma_start(
                            out=out_d[CW * j + 128 * t:CW * j + 128 * (t + 1),
                                      512 * n:512 * (n + 1)],
                            in_=ob)

    nc.compile()
    return nc


def _prep_core_inputs(x, wq, wk, wv, wo, cos2, sin2, maskt, core):
    b, g4 = core // 4, core % 4
    qh0, kv0 = QH * g4, KVH * g4
    deint = np.concatenate([np.arange(0, HD, 2), np.arange(1, HD, 2)])

    xb = np.ascontiguousarray(x[b].T).astype(_BF16)          # [D, S]
    xt = xb.reshape(KT, 128, NCHUNK, CW).transpose(2, 1, 0, 3)  # [chunk, d, ktile, c]
    xt = np.ascontiguousarray(xt)

    wqs = wq[:, qh0 * HD:(qh0 + QH) * HD].reshape(D, QH, HD)[:, :, deint]
    wqs = wqs.reshape(D, QH * HD).astype(_BF16)              # de-interleaved [D, 1024]
    # [pass, k-pair, partition, (k01, cols)] with 256KB contiguous per DMA tile
    wqt = wqs.reshape(KT // 2, 2, 128, 2, 512).transpose(3, 0, 2, 1, 4)
    wqt = np.ascontiguousarray(wqt.reshape(2, KT // 2, 128, 1024))

    wks = wk[:, kv0 * HD:(kv0 + KVH) * HD].reshape(D, KVH, HD)[:, :, deint]
    wks = wks.reshape(D, KVH * HD).astype(_BF16)
    wkt = np.ascontiguousarray(wks.reshape(KT, 128, 256).transpose(1, 0, 2))

    wvs = wv[:, kv0 * HD:(kv0 + KVH) * HD].astype(_BF16)
    wvt = np.ascontiguousarray(wvs.reshape(KT, 128, 256).transpose(1, 0, 2))

    wos = wo[qh0 * HD:(qh0 + QH) * HD, :].astype(_BF16)      # [1024, D]
    wot = np.ascontiguousarray(wos.reshape(QH, 128, 8, 512).transpose(2, 1, 0, 3))

    return {
        "xt": xt, "wq": wqt, "wk": wkt, "wv": wvt, "wo": wot,
        "cos2": cos2, "sin2": sin2, "maskt": maskt,
    }


def kernel(x, wq, wk, wv, wo, start_pos=0, inference=0, _trace=False, **_unused):
    from concourse.bass_utils import run_bass_kernel_spmd

    x = np.asarray(x, np.float32)
    wq = np.asarray(wq, np.float32)
    wk = np.asarray(wk, np.float32)
    wv = np.asarray(wv, np.float32)
    wo = np.asarray(wo, np.float32)

    inv = 1.0 / (THETA ** (np.arange(0, HD, 2, dtype=np.float32) / HD))
    t = np.arange(S, dtype=np.float32)
    ang = np.outer(t, inv).astype(np.float32)                # [S, HD/2]
    cosT = np.cos(ang).T.astype(np.float32)                  # [64, S]
    sinT = np.sin(ang).T.astype(np.float32)
    cos2 = np.ascontiguousarray(np.concatenate([cosT, cosT], 0).astype(_BF16))
    sin2 = np.ascontiguousarray(np.concatenate([sinT, sinT], 0).astype(_BF16))
    kk = np.arange(128)
    maskt = np.where(kk[:, None] > kk[None, :], np.float32(NEG), np.float32(0.0))
    maskt = np.ascontiguousarray(maskt.astype(np.float32))

    nc = _build()
    in_maps = [
        _prep_core_inputs(x, wq, wk, wv, wo, cos2, sin2, maskt, core)
        for core in range(8)
    ]
    res = run_bass_kernel_spmd(nc, in_maps, core_ids=list(range(8)), trace=_trace)
    LAST_INFO["exec_time_ns"] = res.exec_time_ns
    LAST_INFO["results"] = res

    out = np.empty((B, S, D), np.float32)
    for b in range(B):
        out[b] = res.results[4 * b]["out"]
        for g in range(1, 4):
            out[b] += res.results[4 * b + g]["out"]
    return out


# revision 16
# speedup vs baseline: 1.0200x; 1.0200x over previous
"""Trainium2 Bass kernel for GQA attention block (B=2, S=2048, D=4096, 32 q heads,
8 kv heads, rope, causal softmax, output projection).

Sharding: 8 cores = 2 batches x 4 kv-head-groups. Core i handles batch i//4 and
q heads 8*(i%4)..8*(i%4)+7 (kv heads 2*(i%4), 2*(i%4)+1). Each core computes a
partial [S, D] output (its heads' contribution through wo); the host sums the 4
partials per batch.

Compute runs in bf16 on the TensorEngine (fp32 PSUM accumulation). RoPE is
applied with the head dims de-interleaved (even dims in partitions 0:63, odd in
64:127) so the pair arithmetic is partition-aligned; wq/wk columns are permuted
identically on the host, which leaves all dot products unchanged. Scores are
computed transposed (S^T[k,q]) so the probs feed the PV matmul directly; softmax
skips the max subtraction (scores are bounded ~ +-11 for this distribution) and
the row sum comes from a ones-matmul on the PE, replicated across partitions.
"""

import numpy as np
import ml_dtypes

B, S, D = 2, 2048, 4096
NH, NKV, HD = 32, 8, 128
QH = 8          # q heads per core
KVH = 2         # kv heads per core
NCHUNK = 4      # seq chunks of 512
CW = 512        # chunk width
KT = 32         # k-tiles over D
ST = 16         # seq tiles of 128
ISQ = 1.0 / np.sqrt(HD)
THETA = 10000.0
NEG = -1e10

_BF16 = ml_dtypes.bfloat16

LAST_INFO = {}


def _build(trace=False):
    import concourse.bass as bass
    import concourse.mybir as mybir
    from concourse import bacc
    from concourse.tile import TileContext

    f32 = mybir.dt.float32
    bf16 = mybir.dt.bfloat16
    AF = mybir.ActivationFunctionType

    nc = bacc.Bacc("TRN2", target_bir_lowering=False, debug=False, num_devices=8)

    xt_d = nc.dram_tensor("xt", [NCHUNK, 128, KT, CW], bf16, kind="ExternalInput")
    wq_d = nc.dram_tensor("wq", [2, KT // 2, 128, 1024], bf16, kind="ExternalInput")
    wk_d = nc.dram_tensor("wk", [128, KT, 256], bf16, kind="ExternalInput")
    wv_d = nc.dram_tensor("wv", [128, KT, 256], bf16, kind="ExternalInput")
    wo_d = nc.dram_tensor("wo", [8, 128, 8, 512], bf16, kind="ExternalInput")
    cos_d = nc.dram_tensor("cos2", [128, S], bf16, kind="ExternalInput")
    sin_d = nc.dram_tensor("sin2", [128, S], bf16, kind="ExternalInput")
    mask_d = nc.dram_tensor("maskt", [128, 128], f32, kind="ExternalInput")
    out_d = nc.dram_tensor("out", [S, D], f32, kind="ExternalOutput")

    with TileContext(nc) as tc:
        with (
            tc.tile_pool(name="singles", bufs=1) as singles,
            tc.tile_pool(name="xtp", bufs=1) as xtp,
            tc.tile_pool(name="wqp", bufs=4) as wqp,
            tc.tile_pool(name="wop", bufs=3) as wop,
            tc.tile_pool(name="qtp", bufs=2) as qtp,
            tc.tile_pool(name="otp", bufs=2) as otp,
            tc.tile_pool(name="esp", bufs=6) as esp,
            tc.tile_pool(name="rtp", bufs=6) as rtp,
            tc.tile_pool(name="rcp", bufs=2) as rcp,
            tc.tile_pool(name="obp", bufs=4) as obp,
            tc.tile_pool(name="psacc", bufs=4, space="PSUM") as psacc,
            tc.tile_pool(name="pss", bufs=4, space="PSUM") as pss,
        ):
            xt_first = xtp.tile([128, KT, CW], bf16, tag="xt", name="xt_first")
            nc.sync.dma_start(out=xt_first, in_=xt_d[0, :, :, :])
            wk_sb = singles.tile([128, KT, 256], bf16, tag="wk")
            nc.sync.dma_start(out=wk_sb, in_=wk_d[:, :, :])
            wv_sb = singles.tile([128, KT, 256], bf16, tag="wv")
            nc.sync.dma_start(out=wv_sb, in_=wv_d[:, :, :])
            cos_sb = singles.tile([128, S], bf16, tag="cos")
            nc.sync.dma_start(out=cos_sb, in_=cos_d[:, :])
            sin_sb = singles.tile([128, S], bf16, tag="sin")
            nc.sync.dma_start(out=sin_sb, in_=sin_d[:, :])
            mask_sb = singles.tile([128, 128], f32, tag="mask")
            nc.sync.dma_start(out=mask_sb, in_=mask_d[:, :])
            ones_sb = singles.tile([128, 128], bf16, tag="ones")
            nc.vector.memset(ones_sb, 1.0)
            kt_sb = singles.tile([128, KVH, S], bf16, tag="kt")
            v_sb = singles.tile([128, ST, KVH, 128], bf16, tag="v")

            def rope(ps_in, out_ap, j):
                """ps_in: [128, CW] fp32 psum (de-interleaved proj block).
                out_ap: [128, CW] bf16 view <- rope result. The ACT copy frees
                the psum slot immediately; rope math runs in bf16 (DVE 2x)."""
                sl = slice(CW * j, CW * (j + 1))
                qraw = rtp.tile([128, CW], bf16, tag="rt")
                nc.scalar.copy(out=qraw, in_=ps_in)
                tA = rtp.tile([128, CW], bf16, tag="rt")
                tB = rtp.tile([128, CW], bf16, tag="rt")
                # tA: top = x0*cos, bot = x0*sin (inputs base 0)
                nc.vector.tensor_mul(tA[0:64, :], qraw[0:64, :], cos_sb[0:64, sl])
                nc.vector.tensor_mul(tA[64:128, :], qraw[0:64, :], sin_sb[0:64, sl])
                # tB: top = x1*sin, bot = x1*cos (inputs base 64)
                nc.vector.tensor_mul(tB[0:64, :], qraw[64:128, :], sin_sb[64:128, sl])
                nc.vector.tensor_mul(tB[64:128, :], qraw[64:128, :], cos_sb[64:128, sl])
                nc.vector.tensor_sub(out_ap[0:64, :], tA[0:64, :], tB[0:64, :])
                nc.vector.tensor_add(out_ap[64:128, :], tA[64:128, :], tB[64:128, :])

            for j in range(NCHUNK):
                if j == 0:
                    xt_t = xt_first
                else:
                    xt_t = xtp.tile([128, KT, CW], bf16, tag="xt", name=f"xt{j}")
                    nc.sync.dma_start(out=xt_t, in_=xt_d[j, :, :, :])

                # ---- K projection ----
                pk = [psacc.tile([128, CW], f32, tag="acc", name=f"pk{j}_{g}") for g in range(KVH)]
                for k in range(KT):
                    st, sp = (k == 0), (k == KT - 1)
                    for g in range(KVH):
                        nc.tensor.matmul(
                            pk[g], lhsT=wk_sb[:, k, g * 128:(g + 1) * 128],
                            rhs=xt_t[:, k, :], start=st, stop=sp)
                for g in range(KVH):
                    rope(pk[g], kt_sb[:, g, CW * j:CW * (j + 1)], j)

                # ---- Q passes interleaved with V halves (ropes hide under MMs) ----
                qt_t = qtp.tile([128, QH, CW], bf16, tag="qt")

                def q_pass(p):
                    pq = [psacc.tile([128, CW], f32, tag="acc", name=f"pq{j}_{p}_{m}")
                          for m in range(4)]
                    for kp in range(KT // 2):
                        wq_t = wqp.tile([128, 1024], bf16, tag="wq", name=f"wq{j}_{p}_{kp}")
                        nc.sync.dma_start(out=wq_t, in_=wq_d[p, kp, :, :])
                        for k01 in range(2):
                            k = 2 * kp + k01
                            for mm in range(4):
                                nc.tensor.matmul(
                                    pq[mm],
                                    lhsT=wq_t[:, k01 * 512 + mm * 128:k01 * 512 + (mm + 1) * 128],
                                    rhs=xt_t[:, k, :], start=(k == 0), stop=(k == KT - 1))
                    for mm in range(4):
                        rope(pq[mm], qt_t[:, 4 * p + mm, :], j)

                def v_pass(half):
                    pv = [psacc.tile([128, CW], f32, tag="acc", name=f"pv{j}_{half}_{t}")
                          for t in range(2)]
                    for k in range(KT):
                        st, sp = (k == 0), (k == KT - 1)
                        for t in range(2):
                            tt = 2 * half + t
                            nc.tensor.matmul(
                                pv[t][:, 0:256], lhsT=xt_t[:, k, tt * 128:(tt + 1) * 128],
                                rhs=wv_sb[:, k, :], start=st, stop=sp)
                    for t in range(2):
                        nc.scalar.copy(out=v_sb[:, 4 * j + 2 * half + t, :, :],
                                       in_=pv[t][:, 0:256])

                q_pass(0)
                v_pass(0)
                q_pass(1)
                v_pass(1)

                # ---- attention for this chunk's queries ----
                ot_t = otp.tile([128, QH, CW], bf16, tag="ot")
                nk = 4 * j + 4
                for h in range(QH):
                    g = h // 4
                    po = psacc.tile([128, CW], f32, tag="acc")
                    pr = psacc.tile([128, CW], f32, tag="acc")
                    for i in range(nk):
                        off = max(0, 128 * (i - 4 * j))
                        ps = pss.tile([128, CW], f32, tag="s")
                        nc.tensor.matmul(
                            ps[:, off:], lhsT=kt_sb[:, g, 128 * i:128 * (i + 1)],
                            rhs=qt_t[:, h, off:], start=True, stop=True)
                        if i >= 4 * j:
                            nc.vector.tensor_add(
                                ps[:, off:off + 128], ps[:, off:off + 128], mask_sb)
                        es = esp.tile([128, CW], bf16, tag="es")
                        nc.scalar.activation(es[:, off:], ps[:, off:], AF.Exp, scale=ISQ)
                        nc.tensor.matmul(
                            po[:, off:], lhsT=v_sb[:, i, g, :], rhs=es[:, off:],
                            start=(i == 0), stop=(i == nk - 1))
                        nc.tensor.matmul(
                            pr[:, off:], lhsT=ones_sb, rhs=es[:, off:],
                            start=(i == 0), stop=(i == nk - 1))
                    rc = rcp.tile([128, CW], f32, tag="rc")
                    nc.vector.reciprocal_approx_fast(out=rc, in_=pr)
                    nc.vector.tensor_mul(ot_t[:, h, :], po, rc)

                # ---- output projection for this chunk's rows ----
                wo_tiles = []
                for n in range(8):
                    wo_t = wop.tile([128, 8, 512], bf16, tag="wo", name=f"wo{j}_{n}")
                    nc.sync.dma_start(out=wo_t, in_=wo_d[n, :, :, :])
                    wo_tiles.append(wo_t)
                for n in range(8):
                    wo_t = wo_tiles[n]
                    for t in range(4):
                        pw = psacc.tile([128, 512], f32, tag="acc")
                        for hb in range(QH):
                            nc.tensor.matmul(
                                pw, lhsT=ot_t[:, hb, 128 * t:128 * (t + 1)],
                                rhs=wo_t[:, hb, :], start=(hb == 0), stop=(hb == QH - 1))
                        ob = obp.tile([128, 512], f32, tag="ob")
                        nc.vector.tensor_copy(ob, pw)
                        nc.sync.dma_start(
                            out=out_d[CW * j + 128 * t:CW * j + 128 * (t + 1),
                                      512 * n:512 * (n + 1)],
                            in_=ob)

    nc.compile()
    return nc


def _prep_core_inputs(x, wq, wk, wv, wo, cos2, sin2, maskt, core):
    b, g4 = core // 4, core % 4
    qh0, kv0 = QH * g4, KVH * g4
    deint = np.concatenate([np.arange(0, HD, 2), np.arange(1, HD, 2)])

    xb = np.ascontiguousarray(x[b].T).astype(_BF16)          # [D, S]
    xt = xb.reshape(KT, 128, NCHUNK, CW).transpose(2, 1, 0, 3)  # [chunk, d, ktile, c]
    xt = np.ascontiguousarray(xt)

    wqs = wq[:, qh0 * HD:(qh0 + QH) * HD].reshape(D, QH, HD)[:, :, deint]
    wqs = wqs.reshape(D, QH * HD).astype(_BF16)              # de-interleaved [D, 1024]
    # [pass, k-pair, partition, (k01, cols)] with 256KB contiguous per DMA tile
    wqt = wqs.reshape(KT // 2, 2, 128, 2, 512).transpose(3, 0, 2, 1, 4)
    wqt = np.ascontiguousarray(wqt.reshape(2, KT // 2, 128, 1024))

    wks = wk[:, kv0 * HD:(kv0 + KVH) * HD].reshape(D, KVH, HD)[:, :, deint]
    wks = wks.reshape(D, KVH * HD).astype(_BF16)
    wkt = np.ascontiguousarray(wks.reshape(KT, 128, 256).transpose(1, 0, 2))

    wvs = wv[:, kv0 * HD:(kv0 + KVH) * HD].astype(_BF16)
    wvt = np.ascontiguousarray(wvs.reshape(KT, 128, 256).transpose(1, 0, 2))

    wos = wo[qh0 * HD:(qh0 + QH) * HD, :].astype(_BF16)      # [1024, D]
    wot = np.ascontiguousarray(wos.reshape(QH, 128, 8, 512).transpose(2, 1, 0, 3))

    return {
        "xt": xt, "wq": wqt, "wk": wkt, "wv": wvt, "wo": wot,
        "cos2": cos2, "sin2": sin2, "maskt": maskt,
    }


def kernel(x, wq, wk, wv, wo, start_pos=0, inference=0, _trace=False, **_unused):
    from concourse.bass_utils import run_bass_kernel_spmd

    x = np.asarray(x, np.float32)
    wq = np.asarray(wq, np.float32)
    wk = np.asarray(wk, np.float32)
    wv = np.asarray(wv, np.float32)
    wo = np.asarray(wo, np.float32)

    inv = 1.0 / (THETA ** (np.arange(0, HD, 2, dtype=np.float32) / HD))
    t = np.arange(S, dtype=np.float32)
    ang = np.outer(t, inv).astype(np.float32)                # [S, HD/2]
    cosT = np.cos(ang).T.astype(np.float32)                  # [64, S]
    sinT = np.sin(ang).T.astype(np.float32)
    cos2 = np.ascontiguousarray(np.concatenate([cosT, cosT], 0).astype(_BF16))
    sin2 = np.ascontiguousarray(np.concatenate([sinT, sinT], 0).astype(_BF16))
    kk = np.arange(128)
    maskt = np.where(kk[:, None] > kk[None, :], np.float32(NEG), np.float32(0.0))
    maskt = np.ascontiguousarray(maskt.astype(np.float32))

    nc = _build()
    in_maps = [
        _prep_core_inputs(x, wq, wk, wv, wo, cos2, sin2, maskt, core)
        for core in range(8)
    ]
    res = run_bass_kernel_spmd(nc, in_maps, core_ids=list(range(8)), trace=_trace)
    LAST_INFO["exec_time_ns"] = res.exec_time_ns
    LAST_INFO["results"] = res

    out = np.empty((B, S, D), np.float32)
    for b in range(B):
        out[b] = res.results[4 * b]["out"]
        for g in range(1, 4):
            out[b] += res.results[4 * b + g]["out"]
    return out
